# revision 12
# baseline (speedup 1.0000x reference)
"""Trainium2 Bass kernel for ExtractRelevantPatches (pool -> top-k -> gather).

Full-input contract: kernel(heatmap [64,448,448,1] f32, image [64,448,448,3] f32)
-> [1344, 64, 64, 3] f32.

Sharding: pure data-parallel over batch; 8 batches per NeuronCore, 8 cores.

Per-core algorithm (raw Bass, explicit semaphores), v3 — pipelined over 2
groups of 4 batches, whole top-k/index chain on-chip:

  Per group g (batches 4g..4g+3, local b' = 2*bp + par):
  1. Heatmap -> SBUF [128, 2, 7, 448], partition p = 64*par + r. The two
     partition halves ride DIFFERENT HWDGE rings (par0 on SP/sync, par1 on
     ACT/scalar) so both the even- and odd-port halves of the SBUF fabric
     stream concurrently (a single 64-partition DMA only reaches half the
     AXI ports ~217 GB/s).
  2. DVE reduce_sum over 64-col groups -> red [128, 2, 7, 7].
  3. Two accumulating matmuls with selector F_bp [128, 4]
     (F_bp[p, i] = 1 iff i == 2*bp + p//64) -> PSUM psV [4, 49] = per-batch
     pooled sums. No DRAM shuffle.
  4. Top-24 via 3x (max + max_index + match_replace); keep first 21.
  5. base[b',k] = idx + 441*(idx//7) (patch-row units), cast to f32.
  6. 4 broadcast matmuls (lhsT = one-hot row selector E4 [4, 128]) spread
     base to all 128 partitions -> PSUM psD [128, 84]; single DVE
     tensor_tensor add (x4 stride-0 broadcast) with f32 static table
     (7*(p%16) + 112*q + 3136*b) -> int16 idx16 slice [128, 336].
  7. dma_gather chunks on 4 SWDGE queues with graduated sizes
     (128,256,512,896x5 idxs) so the first data flows ~1us after the
     index list instead of ~7.6us (desc-gen runs ~8.5ns/idx on one Q7
     core per chunk; 4 queues give 4-way desc-gen overlap).
  8. One store per gather chunk on the SP HWDGE ring (free after the
     par0 loads), AP out[(c p) e -> p c e].
"""

import numpy as np

_N_CORES = 8
_B = 64
_B_LOC = _B // _N_CORES  # 8
_PATCH = 64
_GRID = 7
_NPATCH = 21
_PROW = _PATCH * 3            # 192 elements per patch-row
_OUT_ROWS_LOC = _B_LOC * _NPATCH  # 168
_NGRP = 2                     # batch groups per core
_BG = _B_LOC // _NGRP         # 4 batches per group

# gather chunk sizes per group, in indices (multiples of 128; sum = 5376)
_CHUNKS = [128, 256, 512, 896, 896, 896, 896, 896]
_NCH_G = len(_CHUNKS)
_NCHUNK = _NGRP * _NCH_G

_nc_cache = None


def build_program():
    """Build the per-core SPMD Bass program (cached)."""
    global _nc_cache
    if _nc_cache is not None:
        return _nc_cache

    import concourse.bass as bass
    import concourse.bacc as bacc
    import concourse.mybir as mybir

    f32 = mybir.dt.float32
    i16 = mybir.dt.int16
    i32 = mybir.dt.int32
    u32 = mybir.dt.uint32
    X = mybir.AxisListType.X
    Op = mybir.AluOpType

    nc = bacc.Bacc(num_swdge_queues=4)

    hm_in = nc.declare_dram_parameter(
        "heatmap", [_B_LOC, 448, 448, 1], f32, isOutput=False)
    img_in = nc.declare_dram_parameter(
        "image", [_B_LOC, 448, 448, 3], f32, isOutput=False)
    out_t = nc.declare_dram_parameter(
        "out", [_OUT_ROWS_LOC, _PATCH, _PATCH, 3], f32, isOutput=True)

    # --- inline constants -------------------------------------------------
    # Static part of the gather index list, f32 [16, 672] tiled x8:
    # position i = R sits at [R%16, R//16]; col s = R//16;
    # static term = 7*(R%16) + 112*(s%4) + 3136*(s//84)
    s_ar = np.arange(672, dtype=np.int64)
    w_ar = np.arange(16, dtype=np.int64)
    st = (112 * (s_ar[None, :] % 4) + 7 * w_ar[:, None]
          + 3136 * (s_ar[None, :] // 84)).astype(np.float32)
    st = np.tile(st, (8, 1))  # replicate across the 8 gpsimd cores
    sttab_const = nc.inline_tensor(st, name="sttab_const")

    # Pooling selectors F_bp [128, 4] packed as one [128, 8]:
    # F[p, 4*bp + i] = 1 iff i == 2*bp + p//64
    F_np = np.zeros((128, 8), dtype=np.float32)
    F_np[:64, 0] = 1.0   # bp=0, b'=0
    F_np[64:, 1] = 1.0   # bp=0, b'=1
    F_np[:64, 6] = 1.0   # bp=1, b'=2
    F_np[64:, 7] = 1.0   # bp=1, b'=3
    F_const = nc.inline_tensor(F_np, name="F_const")

    # Broadcast selectors E4 [4, 512]: E4[p, 128*bl + i] = 1 iff p == bl
    E_np = np.zeros((4, 512), dtype=np.float32)
    for bl in range(4):
        E_np[bl, 128 * bl:128 * (bl + 1)] = 1.0
    E4_const = nc.inline_tensor(E_np, name="E4_const")

    # --- DRAM views -------------------------------------------------------
    # image patch-row view: [25088, 192]
    img_rows = (img_in[:]
                .rearrange("b r c ch -> (b r c ch)")
                .rearrange("(n e) -> n e", e=_PROW))

    # output patch-row view [10752, 192] -> [p, c, e] with R = 128*c + p
    out_pc = (out_t[:]
              .rearrange("r a b c -> (r a b c)")
              .rearrange("(n e) -> n e", e=_PROW)
              .rearrange("(c p) e -> p c e", p=128))

    # heatmap per-group views: [par, 64, 2, 7, 448]
    hm_src = []
    for g in range(_NGRP):
        hm_src.append(
            hm_in[4 * g:4 * (g + 1)]
            .rearrange("(bp par) (br r) c one -> par r bp br (c one)",
                       bp=2, par=2, r=64))

    # per-chunk geometry: (group, idx16 col offset, idx16 col width,
    #                      GT col offset, GT col width, num idxs)
    chunk_geo = []
    for g in range(_NGRP):
        off16 = 336 * g
        offGT = 42 * g
        for n in _CHUNKS:
            chunk_geo.append((g, off16, n // 16, offGT, n // 128, n))
            off16 += n // 16
            offGT += n // 128

    from contextlib import ExitStack

    with ExitStack() as ctx:
        e = ctx.enter_context
        hm = [e(nc.sbuf_tensor(f"hm{g}", [128, 2, 7, 448], f32))
              for g in range(_NGRP)]
        red = [e(nc.sbuf_tensor(f"red{g}", [128, 2, 7, 7], f32))
               for g in range(_NGRP)]
        F_sb = e(nc.sbuf_tensor("F_sb", [128, 8], f32))
        E4_sb = e(nc.sbuf_tensor("E4_sb", [4, 512], f32))
        sttab = e(nc.sbuf_tensor("sttab", [128, 672], f32))
        V = [e(nc.sbuf_tensor(f"V{g}", [4, 49], f32)) for g in range(_NGRP)]
        vwork = [e(nc.sbuf_tensor(f"vwork{g}", [4, 49], f32))
                 for g in range(_NGRP)]
        m8 = [e(nc.sbuf_tensor(f"m8_{g}", [4, 8], f32)) for g in range(_NGRP)]
        idx_u = [e(nc.sbuf_tensor(f"idx_u{g}", [4, 24], u32))
                 for g in range(_NGRP)]
        idx_i = [e(nc.sbuf_tensor(f"idx_i{g}", [4, _NPATCH], i32))
                 for g in range(_NGRP)]
        br_i = [e(nc.sbuf_tensor(f"br_i{g}", [4, _NPATCH], i32))
                for g in range(_NGRP)]
        base_bk = [e(nc.sbuf_tensor(f"base_bk{g}", [4, _NPATCH], i32))
                   for g in range(_NGRP)]
        base_f = [e(nc.sbuf_tensor(f"base_f{g}", [4, _NPATCH], f32))
                  for g in range(_NGRP)]
        idx16 = e(nc.sbuf_tensor("idx16", [128, 672], i16))
        GT = e(nc.sbuf_tensor("GT", [128, 84, _PROW], f32))
        psV = [e(nc.psum_tensor(f"psV{g}", [4, 49], f32))
               for g in range(_NGRP)]
        psD = [e(nc.psum_tensor(f"psD{g}", [128, 84], f32))
               for g in range(_NGRP)]

        s_ld = [[e(nc.semaphore(f"s_ld{g}_{q}")) for q in range(2)]
                for g in range(_NGRP)]
        s_red = [e(nc.semaphore(f"s_red{g}")) for g in range(_NGRP)]
        s_mmV = [e(nc.semaphore(f"s_mmV{g}")) for g in range(_NGRP)]
        s_base = [e(nc.semaphore(f"s_base{g}")) for g in range(_NGRP)]
        s_mmD = [e(nc.semaphore(f"s_mmD{g}")) for g in range(_NGRP)]
        s_idx = [e(nc.semaphore(f"s_idx{g}")) for g in range(_NGRP)]
        s_cst = e(nc.semaphore("s_cst"))
        s_gq = [e(nc.semaphore(f"s_gq{i}")) for i in range(_NCHUNK)]
        s_st = e(nc.semaphore("s_st"))
        block = e(nc.Block())

        @block.sync
        def _(sync):
            # par0 heatmap halves on the SP ring, then the stores
            for g in range(_NGRP):
                for bp in range(2):
                    sync.dma_start(
                        out=hm[g][0:64, bp:bp + 1, :, :],
                        in_=hm_src[g][0:1, :, bp:bp + 1, :, :]
                        .rearrange("one r one2 br c -> (one r) one2 br c"),
                    ).then_inc(s_ld[g][bp], 16)
            for c, (g, o16, w16, oGT, wGT, n) in enumerate(chunk_geo):
                sync.wait_ge(s_gq[c], 16)
                sync.dma_start(
                    out=out_pc[:, oGT:oGT + wGT, :],
                    in_=GT[:, oGT:oGT + wGT, :],
                ).then_inc(s_st, 16)
            sync.wait_ge(s_st, 16 * _NCHUNK)

        @block.scalar
        def _(sc):
            # consts then par1 heatmap halves on the ACT ring
            sc.dma_start(out=F_sb[:], in_=F_const[:]).then_inc(s_cst, 16)
            sc.dma_start(out=E4_sb[:], in_=E4_const[:]).then_inc(s_cst, 16)
            sc.dma_start(out=sttab[:], in_=sttab_const[:]).then_inc(s_cst, 16)
            for g in range(_NGRP):
                for bp in range(2):
                    sc.dma_start(
                        out=hm[g][64:128, bp:bp + 1, :, :],
                        in_=hm_src[g][1:2, :, bp:bp + 1, :, :]
                        .rearrange("one r one2 br c -> (one r) one2 br c"),
                    ).then_inc(s_ld[g][bp], 16)

        @block.vector
        def _(vector):
            for g in range(_NGRP):
                # column-group reduce per batch pair as it lands
                for bp in range(2):
                    vector.wait_ge(s_ld[g][bp], 32)
                    vector.reduce_sum(
                        out=red[g][:, bp:bp + 1, :, :],
                        in_=hm[g][:, bp:bp + 1, :, :].rearrange(
                            "p one br (bc u) -> p (one br bc) u", u=64),
                        axis=X,
                    )
                vector.drain().then_inc(s_red[g], 1)
                # PSUM -> SBUF pooled sums
                vector.wait_ge(s_mmV[g], 1)
                vector.tensor_copy(out=V[g][:], in_=psV[g][:])
                vector.drain()
                # top-24, keep 21
                cur = V[g]
                for r3 in range(3):
                    vector.max(out=m8[g][:], in_=cur[:])
                    vector.drain()
                    vector.max_index(
                        out=idx_u[g][:, 8 * r3:8 * r3 + 8], in_max=m8[g][:],
                        in_values=cur[:])
                    if r3 < 2:
                        nxt = vwork[g] if r3 == 0 else V[g]
                        vector.match_replace(
                            out=nxt[:], in_to_replace=m8[g][:],
                            in_values=cur[:], imm_value=-1e30)
                        vector.drain()
                        cur = nxt
                vector.drain()
                # base = idx + 441*(idx//7); idx//7 via (idx*9363)>>16
                vector.tensor_copy(out=idx_i[g][:], in_=idx_u[g][:, :_NPATCH])
                vector.drain()
                vector.tensor_scalar(
                    out=br_i[g][:], in0=idx_i[g][:], scalar1=9363,
                    scalar2=None, op0=Op.mult)
                vector.drain()
                vector.tensor_scalar(
                    out=br_i[g][:], in0=br_i[g][:], scalar1=16,
                    scalar2=None, op0=Op.logical_shift_right)
                vector.drain()
                vector.tensor_scalar(
                    out=br_i[g][:], in0=br_i[g][:], scalar1=441,
                    scalar2=None, op0=Op.mult)
                vector.drain()
                vector.tensor_tensor(
                    out=base_bk[g][:], in0=idx_i[g][:], in1=br_i[g][:],
                    op=Op.add)
                vector.drain()
                vector.tensor_copy(out=base_f[g][:], in_=base_bk[g][:])
                vector.drain().then_inc(s_base[g], 1)
                # idx16 slice = f32 add of psD (x4 bcast) + static table,
                # cast to i16 on output
                vector.wait_ge(s_mmD[g], 4)
                if g == 0:
                    vector.wait_ge(s_cst, 48)  # sttab is the 3rd const
                vector.tensor_tensor(
                    out=idx16[:, 336 * g:336 * (g + 1)].rearrange(
                        "p (m q) -> p m q", q=4),
                    in0=psD[g][:].rearrange(
                        "p (m one) -> p m one", one=1).to_broadcast(
                        [128, 84, 4]),
                    in1=sttab[:, 336 * g:336 * (g + 1)].rearrange(
                        "p (m q) -> p m q", q=4),
                    op=Op.add)
                vector.drain().then_inc(s_idx[g], 1)

        @block.tensor
        def _(tensor):
            for g in range(_NGRP):
                tensor.wait_ge(s_red[g], 1)
                if g == 0:
                    tensor.wait_ge(s_cst, 16)  # F_sb is the 1st const
                tensor.matmul(
                    out=psV[g][:],
                    lhsT=F_sb[:, 0:4],
                    rhs=red[g][:, 0:1, :, :].rearrange(
                        "p one a b -> p (one a b)"),
                    start=True, stop=False)
                tensor.matmul(
                    out=psV[g][:],
                    lhsT=F_sb[:, 4:8],
                    rhs=red[g][:, 1:2, :, :].rearrange(
                        "p one a b -> p (one a b)"),
                    start=False, stop=True,
                ).then_inc(s_mmV[g], 1)
                # broadcast base across partitions: psD[:, 21b':+21]
                tensor.wait_ge(s_base[g], 1)
                if g == 0:
                    tensor.wait_ge(s_cst, 32)  # E4 is the 2nd const
                for bl in range(4):
                    tensor.matmul(
                        out=psD[g][:, 21 * bl:21 * (bl + 1)],
                        lhsT=E4_sb[:, 128 * bl:128 * (bl + 1)],
                        rhs=base_f[g][:],
                        start=True, stop=True,
                    ).then_inc(s_mmD[g], 1)

        @block.gpsimd
        def _(g_):
            from concourse import library_config
            g_.load_library(library_config.mlp)
            for c, (g, o16, w16, oGT, wGT, n) in enumerate(chunk_geo):
                if c % _NCH_G == 0:
                    g_.wait_ge(s_idx[g], 1)
                g_.dma_gather(
                    out_ap=GT[:, oGT:oGT + wGT, :],
                    in_ap=img_rows,
                    idxs_ap=idx16[:, o16:o16 + w16],
                    num_idxs=n,
                    num_idxs_reg=n,
                    elem_size=_PROW,
                    queue_num=c % 4,
                ).then_inc(s_gq[c], 16)

    nc.finalize()
    _nc_cache = nc
    return nc


def kernel(heatmap, image):
    from concourse.bass_utils import run_bass_kernel_spmd

    heatmap = np.ascontiguousarray(np.asarray(heatmap), dtype=np.float32)
    image = np.ascontiguousarray(np.asarray(image), dtype=np.float32)
    assert heatmap.shape == (_B, 448, 448, 1)
    assert image.shape == (_B, 448, 448, 3)

    nc = build_program()
    in_maps = [
        {
            "heatmap": heatmap[c * _B_LOC:(c + 1) * _B_LOC],
            "image": image[c * _B_LOC:(c + 1) * _B_LOC],
        }
        for c in range(_N_CORES)
    ]
    res = run_bass_kernel_spmd(nc, in_maps, list(range(_N_CORES)))
    outs = [res.results[c]["out"] for c in range(_N_CORES)]
    return np.concatenate(outs, axis=0)
